# revision 55
# baseline (speedup 1.0000x reference)
"""Trainium2 Bass kernel for a dense transformer block (attention + MLP, 2 LayerNorms).

Sharding: data-parallel over 8 cores, one shard per (batch, query-slot-set).
Zigzag query assignment balances causal work: core 2b+0 handles query tiles
{0,3,4,7} of batch b, core 2b+1 handles {1,2,5,6}. Every core computes K/V for
the full 1024-token context from the real x (no zero padding); causal masking
is shipped as per-core data. Score tiles are restricted to the union visibility
qstart = [0,0,128,128,256,256,384,384].

All matmul operands are bf16 (PSUM accumulation fp32); LN statistics and the
softmax denominators are computed in fp32. Output is stored feature-major and
transposed on the host.
"""

from contextlib import ExitStack

import numpy as np
import ml_dtypes

import concourse.bacc as bacc
import concourse.bass as bass
import concourse.tile as tile
from concourse import mybir
from concourse.bass_utils import run_bass_kernel_spmd

B, S, D, H = 4, 1024, 1024, 16
DH = D // H
EPS = 1e-5
TOK = 512   # queries per core
CTX = 1024  # context tokens per core
P = 128
F32 = mybir.dt.float32
F32R = mybir.dt.float32r
BF16 = mybir.dt.bfloat16
AF = mybir.ActivationFunctionType
OP = mybir.AluOpType

N_CORES = 8
QT = [[0, 3, 4, 7], [1, 2, 5, 6]]           # global query tiles per core parity
QSTART = [0, 0, 128, 128, 256, 256, 384, 384]  # first live query col per kt
NPBF = ml_dtypes.bfloat16


def _r(ap):
    """View an fp32 AP as float32r for full-rate PE matmuls."""
    return ap.bitcast(F32R)


def _mm(nc, out, lhsT, rhs, start, stop, tile_position=None):
    nc.tensor.matmul(out, lhsT, rhs, start=start, stop=stop,
                     tile_position=tile_position)


def _bcast_free(ap, n):
    """Insert a stride-0 axis of size n right after the partition dim."""
    return bass.AP(tensor=ap.tensor, offset=ap.offset,
                   ap=[list(ap.ap[0]), [0, n]] + [list(a) for a in ap.ap[1:]])


def build_block_kernel(nc, tc, io):
    ctx = ExitStack()
    (xt2, xq_d, wq_d, wk_d, wv3, bvrow, params_d, wat_d,
     wfc4, wmlp4, maskT, out) = io

    const = ctx.enter_context(tc.tile_pool(name="const", bufs=1))

    ones_bf = const.tile([P, P], BF16)
    nc.vector.memset(ones_bf, 1.0)
    invD = const.tile([P, 1], BF16)
    nc.vector.memset(invD, float(1.0 / D))
    eps_c = const.tile([1, 1], F32)
    nc.vector.memset(eps_c, EPS)

    # ---------------- persistent activations ----------------
    # pools close LIFO: w_pool < xa_pool < v_pool in open order
    w_stack = ExitStack()
    w_pool = w_stack.enter_context(tc.tile_pool(name="w_pool", bufs=1))
    wq_all = w_pool.tile([P, 8, 8, P], BF16)     # [p, hp, dk, m]
    wk_all = w_pool.tile([P, 8, 8, P], BF16)
    wat_all = w_pool.tile([P, 8, 8, P], BF16)    # [p, mt, j, m]
    mask01 = w_pool.tile([P, 8, P], BF16)        # [p(k), kt, q-slot kt//2]

    xa_stack = ExitStack()
    xa_pool = xa_stack.enter_context(tc.tile_pool(name="xa_pool", bufs=1))
    X_f = xa_pool.tile([P, 2, 8, TOK], BF16)     # x^T halves, feature-major
    xq = xa_pool.tile([P, 8, TOK], BF16)         # x^T at own query slots
    a_all = xa_pool.tile([P, 8, TOK], BF16)      # normalized attention out^T

    v_stack = ExitStack()
    v_pool = v_stack.enter_context(tc.tile_pool(name="v_pool", bufs=1))
    V_sb = v_pool.tile([P, 8, H, DH + 1], BF16)  # [V | 1] per head, token-major
    nc.vector.memset(V_sb[:, :, :, DH:DH + 1], 1.0)

    psqk_stack = ExitStack()
    ps_qk = psqk_stack.enter_context(
        tc.tile_pool(name="ps_qk", bufs=2, space="PSUM"))

    # ============ phase 0: stream x / wv / weights, compute V ============
    with tc.tile_pool(name="q_pool", bufs=3) as q_pool, \
            tc.tile_pool(name="k_pool", bufs=3) as k_pool, \
            tc.tile_pool(name="p_pool", bufs=3) as p_pool, \
            tc.tile_pool(name="sm_pool", bufs=3) as sm_pool, \
            tc.tile_pool(name="ps_s", bufs=2, space="PSUM") as ps_s, \
            tc.tile_pool(name="ps_acc", bufs=2, space="PSUM") as ps_acc, \
            tc.tile_pool(name="wv_pool", bufs=1) as wv_pool:
        wv_t = wv_pool.tile([P, 2, 8, TOK], BF16)
        bv_r = const.tile([1, D], BF16)
        # rings in consumption order. V-half1 compute is deferred to mid-
        # attention (first needed by hp4), so only x + wv-h0 are critical.
        nc.scalar.dma_start(out=bv_r, in_=bvrow)
        nc.sync.dma_start(out=X_f[:, 0, 0:4, :], in_=xt2[:, 0, 0:4, :])
        nc.gpsimd.dma_start(out=X_f[:, 0, 4:8, :], in_=xt2[:, 0, 4:8, :])
        nc.scalar.dma_start(out=wv_t[:, 0, 0:4, :], in_=wv3[:, 0, 0:4, :])
        nc.sync.dma_start(out=wv_t[:, 0, 4:6, :], in_=wv3[:, 0, 4:6, :])
        nc.gpsimd.dma_start(out=wv_t[:, 0, 6:8, :], in_=wv3[:, 0, 6:8, :])
        nc.sync.dma_start(out=X_f[:, 1, 0:4, :], in_=xt2[:, 1, 0:4, :])
        nc.gpsimd.dma_start(out=X_f[:, 1, 4:8, :], in_=xt2[:, 1, 4:8, :])
        nc.scalar.dma_start(out=xq, in_=xq_d)
        for c in range(3):
            eng = nc.sync if c % 2 == 0 else nc.gpsimd
            eng2 = nc.gpsimd if c % 2 == 0 else nc.sync
            eng.dma_start(out=wq_all[:, 2 * c:2 * c + 2, :, :],
                          in_=wq_d[:, 2 * c:2 * c + 2, :, :])
            eng2.dma_start(out=wk_all[:, 2 * c:2 * c + 2, :, :],
                           in_=wk_d[:, 2 * c:2 * c + 2, :, :])
        nc.scalar.dma_start(out=mask01, in_=maskT)
        nc.scalar.dma_start(out=wv_t[:, 1, :, :], in_=wv3[:, 1, :, :])
        nc.scalar.dma_start(out=wq_all[:, 6:8, :, :], in_=wq_d[:, 6:8, :, :])
        nc.scalar.dma_start(out=wk_all[:, 6:8, :, :], in_=wk_d[:, 6:8, :, :])

        # all per-partition params arrive in one contiguous [P, 80] DMA:
        # bq | bk | battn | ln1g | ln1b | bmlp | bfc(32)
        pp = const.tile([P, 80], F32)
        nc.sync.dma_start(out=pp, in_=params_d)
        bq_s, bk_s, battn_s = pp[:, 0:8], pp[:, 8:16], pp[:, 16:24]
        ln1g_s, ln1b_s, bmlp_s = pp[:, 24:32], pp[:, 32:40], pp[:, 40:48]
        bfc_s = pp[:, 48:80]
        nc.scalar.dma_start(out=wat_all, in_=wat_d)

        def emit_v2(half, t0, t1):
            for tt in range(t0, t1):
                psV = ps_qk.tile([P, TOK], F32, tag="ps")
                # bias via a K=1 matmul: every token row gets + bv[vf]
                _mm(nc, psV, ones_bf[0:1, :],
                    bv_r[0:1, half * TOK:(half + 1) * TOK],
                    start=True, stop=False)
                for dk in range(8):
                    _mm(nc, psV,
                        X_f[:, tt // 4, dk, (tt % 4) * P:(tt % 4 + 1) * P],
                        wv_t[:, half, dk, :],
                        start=False, stop=(dk == 7))
                # half0 evictions on ACT (idle in phase 0) so the DVE queue
                # reaches hp0's q/k evictions immediately; half1 on DVE
                # (emitted mid-attention where ACT carries the exps)
                if half == 0 and tt % 2 == 0:
                    nc.scalar.activation(
                        V_sb[:, tt, 0:8, 0:DH],
                        psV.rearrange("p (h d) -> p h d", d=DH), AF.Copy)
                else:
                    nc.vector.tensor_copy(
                        out=V_sb[:, tt, half * 8:(half + 1) * 8, 0:DH],
                        in_=psV.rearrange("p (h d) -> p h d", d=DH))

        emit_v2(0, 0, 8)

        # ============== attention, one head-pair at a time ==============
        def qk_start(hp):
            """Begin Q/K projections for head-pair hp (24 MMs, stepped)."""
            return {"hp": hp,
                    "psQ": ps_qk.tile([P, TOK], F32, tag="ps",
                                      name=f"psQ{hp}"),
                    "q_t": q_pool.tile([P, TOK], BF16, tag="q",
                                       name=f"q_t{hp}"),
                    "k_t": k_pool.tile([P, CTX], BF16, tag="k",
                                       name=f"k_t{hp}")}

        def qk_step(st, i):
            """Emit the i-th of 24 Q/K matmuls — PE filler between S pairs."""
            hp = st["hp"]
            if i < 8:
                _mm(nc, st["psQ"], wq_all[:, hp, i, :], xq[:, i, :],
                    start=(i == 0), stop=(i == 7))
                if i == 7:
                    nc.vector.tensor_scalar_add(
                        out=st["q_t"], in0=st["psQ"],
                        scalar1=bq_s[:, hp:hp + 1])
            else:
                half, dk = (i - 8) // 8, (i - 8) % 8
                if dk == 0:
                    st["psK"] = ps_qk.tile([P, TOK], F32, tag="ps",
                                           name=f"psK{st['hp']}_{half}")
                _mm(nc, st["psK"], wk_all[:, hp, dk, :], X_f[:, half, dk, :],
                    start=(dk == 0), stop=(dk == 7))
                if dk == 7:
                    nc.vector.tensor_scalar_add(
                        out=st["k_t"][:, half * TOK:(half + 1) * TOK],
                        in0=st["psK"], scalar1=bk_s[:, hp:hp + 1])

        def emit_S(hp, q_t, k_t, pA, kt):
            qs = QSTART[kt]
            psS = ps_s.tile([P, 2, TOK], F32, tag="s")
            _mm(nc, psS[:, 0, qs:], k_t[0:64, kt * P:(kt + 1) * P],
                q_t[0:64, qs:], start=True, stop=True, tile_position=(0, 0))
            _mm(nc, psS[:, 1, qs:], k_t[64:128, kt * P:(kt + 1) * P],
                q_t[64:128, qs:], start=True, stop=True,
                tile_position=(64, 0))
            nc.scalar.activation(pA[:, kt, :, qs:], psS[:, :, qs:], AF.Exp)
            # only query-slot kt//2 (the first live 128 columns) can be
            # partially visible; all later slots are fully visible for both
            # cores of the pair, so they need no mask multiply
            nc.vector.tensor_mul(pA[:, kt, :, qs:qs + P],
                                 pA[:, kt, :, qs:qs + P],
                                 _bcast_free(mask01[:, kt, :], 2))

        def emit_AV(hp, pA, psA, psB, kt):
            qs = QSTART[kt]
            _mm(nc, psA[:, qs:], V_sb[:, kt, 2 * hp, :], pA[:, kt, 0, qs:],
                start=(kt == 0), stop=(kt == 7))
            _mm(nc, psB[:, qs:], V_sb[:, kt, 2 * hp + 1, :],
                pA[:, kt, 1, qs:], start=(kt == 0), stop=(kt == 7))

        def finale(hp, psA, psB):
            # softmax denominators sit in row 64; broadcast them to
            # partitions 0..63 via a K=1 matmul, then multiply by reciprocal.
            den = sm_pool.tile([65, 2, TOK], BF16, tag="den", bufs=2)
            nc.scalar.activation(den[64:65, 0, :], psA[64:65, :], AF.Copy)
            nc.scalar.activation(den[64:65, 1, :], psB[64:65, :], AF.Copy)
            psDA = ps_qk.tile([64, TOK], F32, tag="ps")
            psDB = ps_qk.tile([64, TOK], F32, tag="ps")
            _mm(nc, psDA, ones_bf[64:65, 0:64], den[64:65, 0, :],
                start=True, stop=True)
            _mm(nc, psDB, ones_bf[64:65, 0:64], den[64:65, 1, :],
                start=True, stop=True)
            rb = sm_pool.tile([64, 2, TOK], F32, tag="rb", bufs=2)
            nc.vector.reciprocal_approx_fast(out=rb[:, 0, :], in_=psDA)
            nc.vector.reciprocal_approx_fast(out=rb[:, 1, :], in_=psDB)
            nc.vector.tensor_mul(a_all[0:64, hp, :], psA[0:64, :], rb[:, 0, :])
            btmp = sm_pool.tile([64, TOK], BF16, tag="btmp", bufs=2)
            nc.vector.tensor_mul(btmp, psB[0:64, :], rb[:, 1, :])
            nc.gpsimd.dma_start(out=a_all[64:128, hp, :], in_=btmp)

        # hp+1's 24 Q/K matmuls are woven 3-per-kt between the exp-gated S
        # pairs so the PE stays dense while ACT/DVE work through exp/mask
        st = qk_start(0)
        for i in range(24):
            qk_step(st, i)
        for hp in range(8):
            q_t, k_t = st["q_t"], st["k_t"]
            if hp < 7:
                nst = qk_start(hp + 1)
            pA = p_pool.tile([P, 8, 2, TOK], BF16, tag="p")
            for kt in range(8):
                emit_S(hp, q_t, k_t, pA, kt)
                if hp < 7:
                    for j in range(3 * kt, 3 * kt + 3):
                        qk_step(nst, j)
            if hp == 2:
                emit_v2(1, 0, 4)   # heads 8..15, first needed by hp4
            if hp == 3:
                emit_v2(1, 4, 8)
            psA = ps_acc.tile([65, TOK], F32, tag="acc")
            psB = ps_acc.tile([65, TOK], F32, tag="acc")
            for kt in range(8):
                emit_AV(hp, pA, psA, psB, kt)
            finale(hp, psA, psB)
            if hp < 7:
                st = nst

    v_stack.close()  # V dead after the last a@v

    # preload the sqrt activation table while the PE runs attnproj (Sqrt and
    # Relu share a table set; Exp does not — this hides the 1.3us reload).
    # The input depends on hp7's output so the scheduler can't hoist it
    # before the last Exp.
    sq_warm = const.tile([1, 1], F32)
    nc.scalar.activation(sq_warm, a_all[0:1, 7, 0:1], AF.Sqrt)

    r1_pool = ctx.enter_context(tc.tile_pool(name="r1_pool", bufs=1, side="right"))
    r1 = r1_pool.tile([P, 8, TOK], BF16)

    # ================= attn projection + residual =================
    for mt in range(8):
        psO = ps_qk.tile([P, TOK], F32, tag="ps")
        for j in range(8):
            _mm(nc, psO, wat_all[:, mt, j, :], a_all[:, j, :],
                start=(j == 0), stop=(j == 7))
        nc.vector.scalar_tensor_tensor(
            out=r1[:, mt, :], in0=psO, scalar=battn_s[:, mt:mt + 1],
            in1=xq[:, mt, :], op0=OP.add, op1=OP.add)

    xa_stack.close()  # X', xq, a_all dead
    w_stack.close()   # wq/wk/wat/mask dead

    def layer_norm(src, dst, g_s=None, b_s=None, warm=0):
        """dst = g * (src - mean) / sqrt(std + eps) + b, stats over the 1024
        features (partition direction, 8 tiles). With g_s/b_s None the affine
        is skipped (folded into the host-side unshard for the final LN)."""
        with tc.tile_pool(name="ln_sb", bufs=2) as ln_sb, \
                tc.tile_pool(name="ln_one", bufs=1) as ln_one, \
                tc.tile_pool(name="ps_stat", bufs=2, space="PSUM") as ps_stat, \
                tc.tile_pool(name="ps_bc", bufs=1, space="PSUM") as ps_bc:
            psSum = ps_stat.tile([1, TOK], F32, tag="st")
            psSq = ps_stat.tile([1, TOK], F32, tag="st")
            for mt in range(8):
                _mm(nc, psSum, invD[:, 0:1], src[:, mt, :],
                    start=(mt == 0), stop=(mt == 7))
                sq_t = ln_sb.tile([P, TOK], BF16, tag="sq")
                nc.vector.tensor_mul(sq_t, src[:, mt, :], src[:, mt, :])
                _mm(nc, psSq, invD[:, 0:1], sq_t,
                    start=(mt == 0), stop=(mt == 7))
            # stats arrive pre-divided by D; row math: unbiased var,
            # q = sqrt(std+eps); broadcast rows, reciprocal after broadcast.
            mrow = ln_one.tile([1, 2, TOK], BF16)   # (mean | q)
            t2 = ln_one.tile([1, TOK], F32)
            t3 = ln_one.tile([1, TOK], F32)
            nc.vector.tensor_copy(out=mrow[0:1, 0, :], in_=psSum)
            nc.vector.tensor_mul(t3, mrow[0:1, 0, :], mrow[0:1, 0, :])
            nc.vector.tensor_sub(t2, psSq, t3)
            nc.scalar.activation(t3, t2, AF.Sqrt, scale=float(D / (D - 1.0)))
            nc.scalar.activation(mrow[0:1, 1, :], t3, AF.Sqrt, bias=eps_c[0:1])
            psMR = ps_bc.tile([P, 2, TOK], F32, tag="bc")
            # keep the PE clock warm through the row-math latency: harmless
            # K=1 matmuls pinned behind the last src tile; the real broadcast
            # below overwrites the bank (start=True clears it)
            for _ in range(warm):
                _mm(nc, psMR[:, 0, :], ones_bf[0:1, :], src[0:1, 7, :],
                    start=True, stop=True)
            _mm(nc, psMR[:, 0, :], ones_bf[0:1, :], mrow[0:1, 0, :],
                start=True, stop=True)
            _mm(nc, psMR[:, 1, :], ones_bf[0:1, :], mrow[0:1, 1, :],
                start=True, stop=True)
            mean_b = ln_one.tile([P, TOK], BF16)
            nc.scalar.activation(mean_b, psMR[:, 0, :], AF.Copy)
            rs_f = ln_one.tile([P, TOK], F32)
            nc.vector.reciprocal_approx_fast(out=rs_f, in_=psMR[:, 1, :])
            rs_b = ln_one.tile([P, TOK], BF16)
            nc.vector.tensor_copy(out=rs_b, in_=rs_f)
            for mt in range(8):
                t1 = ln_sb.tile([P, TOK], BF16, tag="t1")
                nc.vector.tensor_sub(t1, src[:, mt, :], mean_b)
                if g_s is None:
                    nc.vector.tensor_mul(dst[:, mt, :], t1, rs_b)
                else:
                    nc.vector.scalar_tensor_tensor(
                        out=dst[:, mt, :], in0=t1, scalar=g_s[:, mt:mt + 1],
                        in1=rs_b, op0=OP.mult, op1=OP.mult)
                    # bias add on ACT (Identity), off the DVE critical path
                    nc.scalar.activation(dst[:, mt, :], dst[:, mt, :],
                                         AF.Identity, bias=b_s[:, mt:mt + 1])

    with tc.tile_pool(name="h1_pool", bufs=1) as h1_pool:
        h1 = h1_pool.tile([P, 8, TOK], BF16)
        layer_norm(r1, h1, ln1g_s, ln1b_s, warm=18)

        # ================= MLP =================
        with tc.tile_pool(name="r2y", bufs=1) as r2y_pool:
            r2 = r2y_pool.tile([P, 8, TOK], BF16)
            with tc.tile_pool(name="m1_pool", bufs=1) as m1_pool, \
                    tc.tile_pool(name="wfc", bufs=8) as wfc_pool, \
                    tc.tile_pool(name="wmlp", bufs=4) as wmlp_pool:
                m1 = m1_pool.tile([P, 32, TOK], BF16)
                for mt in range(32):
                    wfc_t = wfc_pool.tile([P, 8, P], BF16, tag="wfc")
                    eng = (nc.sync, nc.gpsimd, nc.scalar)[mt % 3]
                    eng.dma_start(out=wfc_t, in_=wfc4[mt])
                    psF = ps_qk.tile([P, TOK], F32, tag="ps")
                    for dk in range(8):
                        _mm(nc, psF, wfc_t[:, dk, :], h1[:, dk, :],
                            start=(dk == 0), stop=(dk == 7))
                    # relu(x + b): alternate DVE / ACT to balance engines
                    if mt % 2 == 0:
                        nc.vector.tensor_scalar(
                            out=m1[:, mt, :], in0=psF,
                            scalar1=bfc_s[:, mt:mt + 1], scalar2=0.0,
                            op0=OP.add, op1=OP.max)
                    else:
                        nc.scalar.activation(m1[:, mt, :], psF, AF.Relu,
                                             bias=bfc_s[:, mt:mt + 1],
                                             scale=1.0)
                for mt in range(8):
                    wmlp_t = wmlp_pool.tile([P, 32, P], BF16, tag="wmlp")
                    eng = (nc.sync, nc.gpsimd, nc.scalar)[mt % 3]
                    eng.dma_start(out=wmlp_t, in_=wmlp4[mt])
                    psM = ps_qk.tile([P, TOK], F32, tag="ps")
                    for k4 in range(32):
                        _mm(nc, psM, wmlp_t[:, k4, :], m1[:, k4, :],
                            start=(k4 == 0), stop=(k4 == 31))
                    nc.vector.scalar_tensor_tensor(
                        out=r2[:, mt, :], in0=psM, scalar=bmlp_s[:, mt:mt + 1],
                        in1=h1[:, mt, :], op0=OP.add, op1=OP.add)

            y = r2y_pool.tile([P, 8, TOK], BF16)
            layer_norm(r2, y)    # LN2 affine applied host-side

            # store feature-major; host transposes and applies g2/b2
            out_r = out.rearrange("a p b -> p a b")
            nc.sync.dma_start(out=out_r[:, 0:2, :], in_=y[:, 0:2, :])
            nc.gpsimd.dma_start(out=out_r[:, 2:4, :], in_=y[:, 2:4, :])
            nc.scalar.dma_start(out=out_r[:, 4:6, :], in_=y[:, 4:6, :])
            nc.sync.dma_start(out=out_r[:, 6:8, :], in_=y[:, 6:8, :])

    ctx.close()


_BUILT = None


def _build():
    global _BUILT
    if _BUILT is not None:
        return _BUILT
    nc = bacc.Bacc("TRN2", target_bir_lowering=False, debug=False,
                   enable_asserts=False, num_devices=N_CORES)

    def din(name, shape, dtype=F32):
        return nc.dram_tensor(name, list(shape), dtype, kind="ExternalInput").ap()

    xt2 = din("xt2", (P, 2, 8, TOK), BF16)      # [p, half, dt, m]
    xq_d = din("xq", (P, 8, TOK), BF16)         # [p, dk, q]
    wq_d = din("wq", (P, 8, 8, P), BF16)        # [p, hp, dk, m] (pre-scaled)
    wk_d = din("wk", (P, 8, 8, P), BF16)
    wv3 = din("wv3", (P, 2, 8, TOK), BF16)      # [p, half, dk, m]
    bvrow = din("bvrow", (1, D), BF16)
    params_d = din("params", (P, 80))   # bq|bk|battn|ln1g|ln1b|bmlp|bfc
    wat_d = din("wat", (P, 8, 8, P), BF16)      # [p, mt, j, m]
    wfc4 = din("wfc4", (32, P, 8, P), BF16)     # [mt, p, dk, m]
    wmlp4 = din("wmlp4", (8, P, 32, P), BF16)   # [mt, p, k4, m]
    maskT = din("maskT", (P, 8, P), BF16)       # [p, kt, 128]
    out_h = nc.dram_tensor("out", [8, P, TOK], BF16, kind="ExternalOutput")

    io = [xt2, xq_d, wq_d, wk_d, wv3, bvrow, params_d, wat_d,
          wfc4, wmlp4, maskT, out_h.ap()]
    with tile.TileContext(nc) as tc:
        build_block_kernel(nc, tc, io)
    nc.compile()
    _BUILT = nc
    return nc


def _tile4(w, n_in, n_out):
    """[K, M] weight -> [n_out, P, n_in, P]: t4[mt, p, k, m] = w[k*P+p, mt*P+m]."""
    K, M = w.shape
    assert K == n_in * P and M == n_out * P
    return np.ascontiguousarray(
        w.reshape(n_in, P, n_out, P).transpose(2, 1, 0, 3))


def _in_maps(inputs):
    f32 = lambda a: np.asarray(a, dtype=np.float32)
    bf = lambda a: np.ascontiguousarray(a).astype(NPBF)
    x = f32(inputs["x"])
    w_qkv = f32(inputs["w_qkv"])
    b_qkv = f32(inputs["b_qkv"]).copy()
    scale = np.float32(1.0 / np.sqrt(DH))
    b_qkv[0:D] *= scale
    # weight tiles shared by all cores; [p, mt, k, m] layouts
    wq4 = _tile4(w_qkv[:, 0:D] * scale, 8, 8)            # [hp, p, dk, m]
    wk4 = _tile4(w_qkv[:, D:2 * D], 8, 8)
    wat4 = _tile4(f32(inputs["w_attn_proj"]), 8, 8)
    colp = lambda v: np.asarray(v, np.float32).reshape(-1, P).T  # [P, n]
    params = np.concatenate([
        colp(b_qkv[0:D]), colp(b_qkv[D:2 * D]), colp(inputs["b_attn_proj"]),
        colp(inputs["ln1_g"]), colp(inputs["ln1_b"]),
        colp(inputs["b_mlp_proj"]), colp(inputs["b_fc"]),
    ], axis=1)
    shared = {
        "wq": bf(wq4.transpose(1, 0, 2, 3)),             # [p, hp, dk, m]
        "wk": bf(wk4.transpose(1, 0, 2, 3)),
        "wat": bf(wat4.transpose(1, 0, 2, 3)),
        "wv3": bf(w_qkv[:, 2 * D:].reshape(8, P, 2, TOK).transpose(1, 2, 0, 3)),
        "wfc4": bf(_tile4(f32(inputs["w_fc"]), 8, 32)),
        "wmlp4": bf(_tile4(f32(inputs["w_mlp_proj"]), 32, 8)),
        "bvrow": bf(b_qkv[2 * D:].reshape(1, D)),
        "params": np.ascontiguousarray(params, dtype=np.float32),
    }
    maps = []
    for b in range(B):
        xT = np.ascontiguousarray(x[b].T)                # [D, S]
        xt2 = bf(xT.reshape(8, P, 2, TOK).transpose(1, 2, 0, 3))
        for hh in range(2):
            gs = QT[hh]
            xqm = np.concatenate([xT[:, g * P:(g + 1) * P] for g in gs], axis=1)
            xq = bf(xqm.reshape(8, P, TOK).transpose(1, 0, 2))
            # mask only covers query-slot kt//2 for each context tile kt
            mask3 = np.zeros((8, P, P), np.float32)
            for kt in range(8):
                g = gs[kt // 2]
                kg = kt * P + np.arange(P)[:, None]
                qg = g * P + np.arange(P)[None, :]
                mask3[kt] = (kg <= qg)
            mask3 = bf(mask3.transpose(1, 0, 2))
            maps.append({"xt2": xt2, "xq": xq, "maskT": mask3, **shared})
    return maps


def run_on_cores(inputs, trace=False, **kwargs):
    """Run the SPMD kernel; returns (full_output, BassKernelResults)."""
    nc = _build()
    maps = _in_maps(inputs)
    res = run_bass_kernel_spmd(nc, maps, core_ids=list(range(N_CORES)),
                               trace=trace, **kwargs)
    g2 = np.asarray(inputs["ln2_g"], np.float32)
    b2 = np.asarray(inputs["ln2_b"], np.float32)
    out = np.zeros((B, S, D), np.float32)
    for c in range(N_CORES):
        b, hh = divmod(c, 2)
        yT = np.asarray(res.results[c]["out"]).astype(np.float32)
        yT = yT.reshape(D, TOK).T * g2[None, :] + b2[None, :]  # [q_local, D]
        for j, g in enumerate(QT[hh]):
            out[b, g * P:(g + 1) * P] = yT[j * P:(j + 1) * P]
    return out, res


def kernel(**inputs) -> np.ndarray:
    out, _ = run_on_cores(inputs, trace=False)
    return out
